# revision 21
# baseline (speedup 1.0000x reference)
"""GRU single-step kernel for Trainium2, data-parallel over 8 NeuronCores.

Computes h_next = GRUCell(x, h_prev) with PyTorch gate layout [r; z; n]:
    gi = x @ W_ih.T + b_ih ; gh = h @ W_hh.T + b_hh
    r = sigmoid(gi_r + gh_r); z = sigmoid(gi_z + gh_z)
    n = tanh(gi_n + r * gh_n); h' = (1-z)*n + z*h

Strategy: shard batch (16384 -> 8 x 2048); weights replicated. Weight-
stationary layout with gates on PSUM partitions and batch on the free dim, so
the per-gate biases fold into the ACT engine's per-partition bias operand.
All matmuls are fp8(e4m3) DoubleRow (hw: 2x bf16 MAC rate, 256 contraction
rows per 512-cycle pass). The dominant quantization error term (n-gate
x-side) gets one extra compensation pass dx@W where dx = fp8(x - fp8(x)) is
the unscaled fp8 residual (subnormals cover its range), accumulated into the
same PSUM bank at the same scale. Measured device rel err 1.675e-2 (gate
2e-2). Weights are pre-scaled by 32 so fp8 stays out of subnormals; the 1/32
folds into the ACT activation scale operand.

Epilogue per (block, j) on [128 gates, 512 batch] tiles:
  ACT: r = sig(R/32 + br), z = sig(Z/32 + bz), n = tanh(NI'/32 + bni)
  DVE: t = (NH + 32*bnh)*r (fused scalar_tensor_tensor), NI' = NI + t
       (in-place PSUM), hm1 = h - n, hm2 = z*hm1, out = n + hm2
All sigmoid/tanh live in one ACT table; a warm-up activation pulls the table
load off the first epilogue's critical path.

PE notes: group order alternates (NH,R,Z,NI / NI,NH,R,Z) so accumulation-
group boundaries pack cleanly and the final group drains through the
shortest chain (ends on Z). Steady state is one DR matmul every 216ns,
gap-free except the hardware power-throttle duty cycle.

DMA notes: issues live only on the SP and Pool-SWDGE queues (DMA issue
instructions can block on recycled DMA semaphores; on a compute queue that
would stall ACT/DVE). Transfers are emitted in first-consumption order; the
output tiles go out via SP HWDGE so the final SWDGE drain has no pending
work.
"""

import os
import sys

import numpy as np

if "/opt/trn_rl_repo" not in sys.path:
    sys.path.insert(0, "/opt/trn_rl_repo")

H = 1024            # hidden == input size
B = 16384
NCORES = 8
BLOC = B // NCORES  # 2048 rows per core
NB = 512            # batch columns per block (PSUM bank width)
NBLK = BLOC // NB   # 4 blocks per core
KP = 4              # fp8 DoubleRow k-pairs (2x128 contraction per pass)
KC = 8              # bf16 k-chunks (128 contraction per pass)
NJ = H // 128       # 8 hidden chunks of 128 gates
S = 32.0            # weight pre-scale

_cache = {}


def _build_program():
    from concourse import bacc, bass, mybir, tile

    f32 = mybir.dt.float32
    bf16 = mybir.dt.bfloat16
    f8 = mybir.dt.float8e4
    Alu = mybir.AluOpType
    ActFn = mybir.ActivationFunctionType
    DR = mybir.MatmulPerfMode.DoubleRow

    nc = bacc.Bacc("TRN2", target_bir_lowering=False, debug=False)

    xm_d = nc.declare_dram_parameter("xm", [NBLK, 128, KP, 2, NB], f8, isOutput=False)
    dxm_d = nc.declare_dram_parameter("dxm", [NBLK, 128, KP, 2, NB], f8, isOutput=False)
    hm_d = nc.declare_dram_parameter("hm", [NBLK, 128, KP, 2, NB], f8, isOutput=False)
    hb_d = nc.declare_dram_parameter("hb", [NBLK, 128, NJ, NB], bf16, isOutput=False)
    wih_d = nc.declare_dram_parameter("wihT", [128, NJ, 3, KP, 2, 128], f8, isOutput=False)
    whh_d = nc.declare_dram_parameter("whhT", [128, NJ, 3, KP, 2, 128], f8, isOutput=False)
    bias_d = nc.declare_dram_parameter("biasT", [128, NJ, 4], f32, isOutput=False)
    out_d = nc.declare_dram_parameter("h_next", [NBLK, 128, NJ, NB], bf16, isOutput=True)

    with tile.TileContext(nc) as tc:
        with (
            tc.tile_pool(name="wpool", bufs=1) as wpool,
            tc.tile_pool(name="stream", bufs=2) as stream,
            tc.tile_pool(name="temps", bufs=4) as temps,
            tc.tile_pool(name="psum", bufs=2, space="PSUM") as psum,
        ):
            wih_t = wpool.tile([128, NJ, 3, KP, 2, 128], f8, tag="wih")
            whh_t = wpool.tile([128, NJ, 3, KP, 2, 128], f8, tag="whh")
            bias_t = wpool.tile([128, NJ, 4], f32, tag="bias")

            xm_ts, xb_ts, hm_ts, hb_ts = [], [], [], []
            for bb in range(NBLK):
                xm_ts.append(stream.tile([128, KP, 2, NB], f8, tag="xm", name=f"xm{bb}"))
                xb_ts.append(stream.tile([128, KP, 2, NB], f8, tag="dxm", bufs=3, name=f"dxm{bb}"))
                hm_ts.append(stream.tile([128, KP, 2, NB], f8, tag="hm", name=f"hm{bb}"))
                hb_ts.append(stream.tile([128, NJ, NB], bf16, tag="hb", bufs=3, name=f"hb{bb}"))

            # Startup DMAs in first-group consumption order. Only the SP and
            # Pool queues carry DMA issues: their issue instructions can block
            # on recycled DMA semaphores, which is harmless there but would
            # stall the ACT/DVE compute queues.
            # The first NH group consumes whh0's n-gate slice then hm0 kp by
            # kp; chunking these lets the PE start ~3us earlier (it runs at
            # low p-state here anyway, so mild transfer stutter is cheap).
            nc.gpsimd.dma_start(out=whh_t[:, 0, 2], in_=whh_d[:, 0, 2])
            for kp in range(KP):
                nc.gpsimd.dma_start(out=hm_ts[0][:, kp], in_=hm_d[0, :, kp])
            nc.sync.dma_start(out=bias_t[:], in_=bias_d[:])
            nc.sync.dma_start(out=wih_t[:, 0], in_=wih_d[:, 0])
            nc.sync.dma_start(out=xm_ts[0][:], in_=xm_d[0])
            nc.gpsimd.dma_start(out=whh_t[:, 0, 0:2], in_=whh_d[:, 0, 0:2])
            nc.gpsimd.dma_start(out=xb_ts[0][:], in_=dxm_d[0])

            def wj(j):
                nc.gpsimd.dma_start(out=whh_t[:, j], in_=whh_d[:, j])
                nc.sync.dma_start(out=wih_t[:, j], in_=wih_d[:, j])

            wj(1)
            nc.gpsimd.dma_start(out=hb_ts[0][:], in_=hb_d[0])
            wj(2)
            wj(3)
            nc.sync.dma_start(out=xm_ts[1][:], in_=xm_d[1])
            nc.gpsimd.dma_start(out=hm_ts[1][:], in_=hm_d[1])
            nc.sync.dma_start(out=xb_ts[1][:], in_=dxm_d[1])
            wj(4)
            nc.sync.dma_start(out=hb_ts[1][:], in_=hb_d[1])
            wj(5)
            wj(6)
            wj(7)

            # Warm-up activation: pulls the sigmoid ACT table load off the
            # first real epilogue's critical path.
            wtmp = temps.tile([128, 1], f32, tag="wtmp", bufs=1)
            nc.vector.memset(wtmp[:], 0.0)
            nc.scalar.activation(wtmp[:], wtmp[:], ActFn.Sigmoid)

            for bb in range(NBLK):
                xm_t, xb_t, hm_t, hb_t = xm_ts[bb], xb_ts[bb], hm_ts[bb], hb_ts[bb]
                if 1 <= bb < NBLK - 1:
                    nc.sync.dma_start(out=xm_ts[bb + 1][:], in_=xm_d[bb + 1])
                    nc.gpsimd.dma_start(out=hm_ts[bb + 1][:], in_=hm_d[bb + 1])
                    nc.sync.dma_start(out=xb_ts[bb + 1][:], in_=dxm_d[bb + 1])
                    nc.sync.dma_start(out=hb_ts[bb + 1][:], in_=hb_d[bb + 1])
                out_t = stream.tile([128, NJ, NB], bf16, tag="out")

                for j in range(NJ):
                    R = psum.tile([128, NB], f32, tag="R")
                    Z = psum.tile([128, NB], f32, tag="Z")
                    NI = psum.tile([128, NB], f32, tag="NI")
                    NH = psum.tile([128, NB], f32, tag="NH")

                    def mm_nh():
                        # n gate, h-side: fp8 DR (frees its bank early via t)
                        for kp in range(KP):
                            nc.tensor.matmul(NH[:], whh_t[:, j, 2, kp], hm_t[:, kp],
                                             start=(kp == 0), stop=(kp == KP - 1), perf_mode=DR)

                    def mm_rz():
                        # r/z gates: fp8 DR, x-side + h-side into one bank
                        for g, P_ in ((0, R), (1, Z)):
                            for kp in range(KP):
                                nc.tensor.matmul(P_[:], wih_t[:, j, g, kp], xm_t[:, kp],
                                                 start=(kp == 0), stop=False, perf_mode=DR)
                            for kp in range(KP):
                                nc.tensor.matmul(P_[:], whh_t[:, j, g, kp], hm_t[:, kp],
                                                 start=False, stop=(kp == KP - 1), perf_mode=DR)

                    def mm_ni():
                        # n gate, x-side: fp8 DR with an unscaled-fp8 dx
                        # residual pass (compensates the dominant error term)
                        for kp in range(KP):
                            nc.tensor.matmul(NI[:], wih_t[:, j, 2, kp], xm_t[:, kp],
                                             start=(kp == 0), stop=False, perf_mode=DR)
                        for kp in range(KP):
                            nc.tensor.matmul(NI[:], wih_t[:, j, 2, kp], xb_t[:, kp],
                                             start=False, stop=(kp == KP - 1), perf_mode=DR)

                    # Alternate the per-group order so adjacent groups' bf16
                    # runs are back-to-back: the PE pays ~190ns per bf16->fp8DR
                    # mode switch, this halves the count (and the last group
                    # ends on Z, shortening the drain chain).
                    if j % 2 == 0:
                        mm_nh(); mm_rz(); mm_ni()
                    else:
                        mm_ni(); mm_nh(); mm_rz()

                    # epilogue
                    r = temps.tile([128, NB], bf16, tag="r")
                    nc.scalar.activation(r[:], R[:], ActFn.Sigmoid,
                                         bias=bias_t[:, j, 0:1], scale=1.0 / S)
                    z = temps.tile([128, NB], bf16, tag="z")
                    nc.scalar.activation(z[:], Z[:], ActFn.Sigmoid,
                                         bias=bias_t[:, j, 1:2], scale=1.0 / S)
                    # t = (NH + 32*bnh) * r
                    t = temps.tile([128, NB], bf16, tag="t")
                    nc.vector.scalar_tensor_tensor(t[:], NH[:], bias_t[:, j, 3:4], r[:],
                                                   Alu.add, Alu.mult)
                    # NI += t (in place, PSUM)
                    nc.vector.tensor_tensor(NI[:], NI[:], t[:], Alu.add)
                    n = temps.tile([128, NB], bf16, tag="n")
                    nc.scalar.activation(n[:], NI[:], ActFn.Tanh,
                                         bias=bias_t[:, j, 2:3], scale=1.0 / S)
                    # h' = n + z*(h - n)
                    hm1 = temps.tile([128, NB], bf16, tag="hm1")
                    nc.vector.tensor_tensor(hm1[:], hb_t[:, j], n[:], Alu.subtract)
                    hm2 = temps.tile([128, NB], bf16, tag="hm2")
                    nc.vector.tensor_tensor(hm2[:], z[:], hm1[:], Alu.mult)
                    nc.vector.tensor_tensor(out_t[:, j], n[:], hm2[:], Alu.add)
                    nc.sync.dma_start(out=out_d[bb, :, j], in_=out_t[:, j])

    nc.compile()
    return nc


def _prep_inputs(x, h_prev, weight_ih, weight_hh, bias_ih, bias_hh):
    import ml_dtypes

    bf16 = ml_dtypes.bfloat16
    f8 = ml_dtypes.float8_e4m3fn if hasattr(ml_dtypes, "float8_e4m3fn") else ml_dtypes.float8_e4m3

    # fp8 moving: [core, blk, p, kp, i, b] = a[core*2048+blk*512+b, (2kp+i)*128+p]
    def to_moving8(a):
        v = a.astype(f8).reshape(NCORES, NBLK, NB, KP, 2, 128).transpose(0, 1, 5, 3, 4, 2)
        return np.ascontiguousarray(v)

    xm = to_moving8(x)
    hm = to_moving8(h_prev)
    # unscaled fp8 residual of x (subnormals cover the small range)
    dxm = to_moving8(x - x.astype(f8).astype(np.float32))
    hb = np.ascontiguousarray(
        h_prev.astype(bf16).reshape(NCORES, NBLK, NB, NJ, 128).transpose(0, 1, 4, 3, 2))

    # fp8 stationary: [pk, j, g, kp, i, mg] = Wq[g*1024 + j*128 + mg, (2kp+i)*128 + pk]
    def to_stationary8(w, ngates):
        v = (S * w).astype(f8).reshape(ngates, NJ, 128, KP, 2, 128).transpose(5, 1, 0, 3, 4, 2)
        return np.ascontiguousarray(v)

    wihT = to_stationary8(weight_ih, 3)
    whhT = to_stationary8(weight_hh, 3)

    bias = np.empty((128, NJ, 4), np.float32)
    bias[:, :, 0] = (bias_ih[:H] + bias_hh[:H]).reshape(NJ, 128).T
    bias[:, :, 1] = (bias_ih[H:2 * H] + bias_hh[H:2 * H]).reshape(NJ, 128).T
    bias[:, :, 2] = bias_ih[2 * H:].reshape(NJ, 128).T
    bias[:, :, 3] = (S * bias_hh[2 * H:]).reshape(NJ, 128).T

    in_maps = []
    for c in range(NCORES):
        in_maps.append({
            "xm": xm[c], "dxm": dxm[c], "hm": hm[c], "hb": hb[c],
            "wihT": wihT, "whhT": whhT, "biasT": bias,
        })
    return in_maps


def kernel(x, h_prev, weight_ih, weight_hh, bias_ih, bias_hh):
    from concourse.bass_utils import run_bass_kernel_spmd

    x = np.asarray(x, dtype=np.float32)
    h_prev = np.asarray(h_prev, dtype=np.float32)
    weight_ih = np.asarray(weight_ih, dtype=np.float32)
    weight_hh = np.asarray(weight_hh, dtype=np.float32)
    bias_ih = np.asarray(bias_ih, dtype=np.float32)
    bias_hh = np.asarray(bias_hh, dtype=np.float32)

    if "nc" not in _cache:
        _cache["nc"] = _build_program()
    nc = _cache["nc"]

    in_maps = _prep_inputs(x, h_prev, weight_ih, weight_hh, bias_ih, bias_hh)
    trace = os.environ.get("GRU_TRACE", "0") == "1"
    res = run_bass_kernel_spmd(nc, in_maps, list(range(NCORES)), trace=trace)
    kernel._last_exec_ns = res.exec_time_ns

    outs = []
    for c in range(NCORES):
        o = np.asarray(res.results[c]["h_next"])  # [NBLK, 128, NJ, NB] bf16
        outs.append(o.transpose(0, 3, 2, 1).reshape(BLOC, H))
    return np.concatenate(outs, axis=0).astype(np.float32)


kernel._last_exec_ns = None
